# Initial kernel scaffold
#
"""ContraNorm kernel for Trainium2 (Bass/Tile), 8-core data-parallel over batch.

Reference computation per batch element b (x_b: [N=2048, D=512] fp32):
    xn   = x_b / ||x_b||_2(rows)
    sim  = xn @ xn.T                      # [N, N]
    P    = softmax(sim, axis=1)
    y    = layernorm(x_b - 0.1 * P @ x_b) * w + b

Key structure exploited: E = exp(sim) is SYMMETRIC, so the second matmul
(P @ x = diag(1/rowsum(E)) . E @ x) can use E's natural-layout tiles as the
stationary (pre-transposed) matmul operand — no on-chip transpose of the
2048x2048 matrix is ever needed. Softmax needs no max-subtraction since
sim entries are cosine similarities in [-1, 1].

Phases (per core):
  0: load x, row sumsq -> invnorm (sqrt table), xn = x*invn (bf16),
     PE-transpose xn -> xnT [D, N] bf16, cast x -> bf16 (ACT)
  A: sim row-blocks via PE (bf16), exp on ACT (psum->sbuf bf16) with
     accum_out giving softmax row-sums for free
  B: x_neg row-blocks via PE using E column-tiles as weights, epilogue
     (residual + layernorm) on DVE, store
"""

import numpy as np
from contextlib import ExitStack

import concourse.bass as bass
import concourse.tile as tile
from concourse import bacc, mybir
from concourse.bass_utils import run_bass_kernel_spmd
from concourse.masks import make_identity

F32 = mybir.dt.float32
BF16 = mybir.dt.bfloat16
AF = mybir.ActivationFunctionType
OP = mybir.AluOpType

B, N, D = 8, 2048, 512
P = 128                 # partitions
NT = N // P             # 16 row blocks
KD = D // P             # 4 d blocks
NC_CHUNK = 512          # matmul free-dim chunk (one PSUM bank)
SCALE = 0.1
EPS = 1e-6


def build_nc(n: int = N, num_devices: int = B, per_block_invn: bool = True,
             gp_cast: bool = True, gp_wb: bool = True):
    NT = n // P
    nc = bacc.Bacc("TRN2", target_bir_lowering=False, debug=False,
                   num_devices=num_devices)

    x_d = nc.dram_tensor("x", [n, D], F32, kind="ExternalInput").ap()
    w_d = nc.dram_tensor("ln_weight", [D], F32, kind="ExternalInput").ap()
    b_d = nc.dram_tensor("ln_bias", [D], F32, kind="ExternalInput").ap()
    out_d = nc.dram_tensor("out", [n, D], F32, kind="ExternalOutput").ap()

    x_r = x_d.rearrange("(t p) d -> p t d", p=P)      # [128, 16, 512]
    out_r = out_d.rearrange("(t p) d -> p t d", p=P)

    with tile.TileContext(nc) as tc, ExitStack() as ctx:
        const = ctx.enter_context(tc.tile_pool(name="const", bufs=1))
        big = ctx.enter_context(tc.tile_pool(name="big", bufs=1))
        work = ctx.enter_context(tc.tile_pool(name="work", bufs=3))
        stats = ctx.enter_context(tc.tile_pool(name="stats", bufs=4))
        outp = ctx.enter_context(tc.tile_pool(name="outp", bufs=3))
        psA = ctx.enter_context(tc.tile_pool(name="psA", bufs=2, space="PSUM"))
        psS = ctx.enter_context(tc.tile_pool(name="psS", bufs=2, space="PSUM"))
        psB = ctx.enter_context(tc.tile_pool(name="psB", bufs=2, space="PSUM"))

        # --- constants ---
        ident = const.tile([P, P], BF16)
        make_identity(nc, ident[:])
        w_bc = const.tile([P, D], F32)
        b_bc = const.tile([P, D], F32)
        nc.gpsimd.dma_start(
            out=w_bc[:], in_=bass.AP(tensor=w_d.tensor, offset=w_d.offset,
                                     ap=[[0, P], w_d.ap[0]]))
        nc.gpsimd.dma_start(
            out=b_bc[:], in_=bass.AP(tensor=b_d.tensor, offset=b_d.offset,
                                     ap=[[0, P], b_d.ap[0]]))

        # --- persistent big buffers ---
        x_all = big.tile([P, NT, D], F32)        # x row blocks      (32KB/part)
        xb_all = big.tile([P, NT, D], BF16)      # x bf16            (16KB/part)
        xnT = big.tile([P, KD, n], BF16)         # xn^T d-blocks     (16KB/part)
        e_all = big.tile([P, NT, n], BF16)       # exp(sim)          (64KB/part)

        HW_ = min(n, 2 * NC_CHUNK)               # psum strip width
        H = n // HW_                             # strips per sim row

        eps_t = const.tile([P, 1], F32)
        nc.vector.memset(eps_t[:], EPS)
        ss = const.tile([P, NT], F32)            # row sum-of-squares
        invn = const.tile([P, NT], F32)          # 1/||row||
        s_half = const.tile([P, H * NT], F32)    # exp accum partials
        rsum = const.tile([P, NT], F32)          # softmax row sums
        negr = const.tile([P, NT], F32)          # -SCALE / rowsum

        # ---------------- Phase 0: load, norms, casts, transposes ----------
        # Per-block dependency chains (no cross-block barrier) so the PE can
        # start sim matmuls as soon as the first 8 blocks are transposed.
        for i in range(NT):
            nc.sync.dma_start(out=x_all[:, i, :], in_=x_r[:, i, :])
            # sum-of-squares via ACT Square's free accumulator (output unused)
            sq_scratch = work.tile([P, D], BF16, tag="sq")
            nc.scalar.activation(out=sq_scratch[:], in_=x_all[:, i, :],
                                 func=AF.Square, accum_out=ss[:, i:i + 1])
            if per_block_invn:
                nc.vector.reciprocal(out=invn[:, i:i + 1], in_=ss[:, i:i + 1])
                nc.scalar.activation(out=invn[:, i:i + 1],
                                     in_=invn[:, i:i + 1],
                                     func=AF.Sqrt)  # 1/||row|| = sqrt(1/ss)
            elif i == NT - 1:
                nc.vector.reciprocal(out=invn[:], in_=ss[:])
                nc.scalar.activation(out=invn[:], in_=invn[:], func=AF.Sqrt)
        for i in range(NT):
            xn_i = work.tile([P, D], BF16, tag="xn")
            nc.vector.tensor_scalar_mul(xn_i[:], x_all[:, i, :], invn[:, i:i + 1])
            pt = psS.tile([P, KD * P], BF16, tag="pt")
            for k in range(KD):
                nc.tensor.transpose(pt[:, k * P:(k + 1) * P],
                                    xn_i[:, k * P:(k + 1) * P], ident[:])
            # pt[:, k*128+m] = xn[i*128+m, k*128+dp] -> xnT[dp, k, i*128+m]
            nc.vector.tensor_copy(
                xnT[:, :, i * P:(i + 1) * P],
                pt[:].rearrange("p (k m) -> p k m", k=KD))
            # x -> bf16 (needed only in phase B)
            if gp_cast:
                nc.gpsimd.tensor_copy(out=xb_all[:, i, :], in_=x_all[:, i, :])
            else:
                nc.scalar.copy(out=xb_all[:, i, :], in_=x_all[:, i, :])

        # ---------------- Phase A: sim blocks + exp ------------------------
        for i in range(NT):
            for h in range(H):                    # psum strips
                pa = psA.tile([P, HW_], F32, tag="pa")
                for c in range(HW_ // NC_CHUNK):  # bank-sized chunks
                    col0 = h * HW_ + c * NC_CHUNK
                    for k in range(KD):
                        nc.tensor.matmul(
                            pa[:, c * NC_CHUNK:(c + 1) * NC_CHUNK],
                            lhsT=xnT[:, k, i * P:(i + 1) * P],
                            rhs=xnT[:, k, col0:col0 + NC_CHUNK],
                            start=(k == 0), stop=(k == KD - 1))
                nc.scalar.activation(
                    out=e_all[:, i, h * HW_:(h + 1) * HW_], in_=pa[:],
                    func=AF.Exp, accum_out=s_half[:, H * i + h:H * i + h + 1])

        # row sums and -SCALE/rowsum
        if H > 1:
            s_view = s_half.rearrange("p (i h) -> p i h", h=H)
            nc.vector.reduce_sum(
                rsum[:].rearrange("p (i o) -> p i o", o=1), s_view,
                axis=mybir.AxisListType.X)
        else:
            nc.vector.tensor_copy(rsum[:], s_half[:])
        nc.vector.reciprocal(out=negr[:], in_=rsum[:])
        nc.vector.tensor_scalar_mul(negr[:], negr[:], -SCALE)

        # ---------------- Phase B: x_neg + residual + layernorm ------------
        for i in range(NT):
            pb = psB.tile([P, D], F32, tag="pb")
            for j in range(NT):
                nc.tensor.matmul(
                    pb[:], lhsT=e_all[:, j, i * P:(i + 1) * P],
                    rhs=xb_all[:, j, :], start=(j == 0), stop=(j == NT - 1))

            t0 = work.tile([P, D], F32, tag="t0")
            nc.vector.tensor_scalar_mul(t0[:], pb[:], negr[:, i:i + 1])
            y0 = work.tile([P, D], F32, tag="y0")
            nc.vector.tensor_add(y0[:], t0[:], x_all[:, i, :])

            # mean/var: DVE row-sum + ACT Square accumulator
            s1 = stats.tile([P, 1], F32, tag="s1")
            nc.vector.reduce_sum(s1[:], y0[:], axis=mybir.AxisListType.X)
            sqd = work.tile([P, D], BF16, tag="sqd")
            ssq = stats.tile([P, 1], F32, tag="ssq")
            nc.scalar.activation(out=sqd[:], in_=y0[:], func=AF.Square,
                                 accum_out=ssq[:])
            mean = stats.tile([P, 1], F32, tag="mean")
            nc.vector.tensor_scalar_mul(mean[:], s1[:], 1.0 / D)
            m2 = stats.tile([P, 1], F32, tag="m2")
            nc.vector.tensor_mul(m2[:], mean[:], mean[:])
            var = stats.tile([P, 1], F32, tag="var")
            nc.vector.tensor_scalar_mul(var[:], ssq[:], 1.0 / D)
            nc.vector.tensor_sub(var[:], var[:], m2[:])
            rstd = stats.tile([P, 1], F32, tag="rstd")
            nc.scalar.activation(out=rstd[:], in_=var[:], func=AF.Sqrt,
                                 bias=eps_t[:], scale=1.0)  # sqrt(var+eps)
            nc.vector.reciprocal(out=rstd[:], in_=rstd[:])

            z0 = work.tile([P, D], F32, tag="z0")
            nc.vector.tensor_scalar_sub(z0[:], y0[:], mean[:])
            z = work.tile([P, D], F32, tag="z")
            nc.vector.tensor_scalar_mul(z[:], z0[:], rstd[:])
            # ln weight/bias application
            eng = nc.gpsimd if gp_wb else nc.vector
            zw = work.tile([P, D], F32, tag="zw")
            eng.tensor_mul(zw[:], z[:], w_bc[:])
            yout = outp.tile([P, D], F32, tag="yout")
            eng.tensor_add(yout[:], zw[:], b_bc[:])
            nc.sync.dma_start(out=out_r[:, i, :], in_=yout[:])

    nc.compile()
    return nc


_NC = None


def _get_nc():
    global _NC
    if _NC is None:
        _NC = build_nc()
    return _NC


def _run(inputs: dict, trace: bool = False, **kw):
    x = np.asarray(inputs["x"], dtype=np.float32)
    w = np.asarray(inputs["ln_weight"], dtype=np.float32)
    bb = np.asarray(inputs["ln_bias"], dtype=np.float32)
    assert x.shape == (B, N, D), x.shape
    nc = _get_nc()
    in_maps = [
        {"x": np.ascontiguousarray(x[b]), "ln_weight": w, "ln_bias": bb}
        for b in range(B)
    ]
    res = run_bass_kernel_spmd(nc, in_maps, core_ids=list(range(B)), trace=trace,
                               **kw)
    out = np.stack([res.results[b]["out"] for b in range(B)], axis=0)
    return out.astype(np.float32), res


def kernel(**inputs) -> np.ndarray:
    out, _ = _run(inputs, trace=False)
    return out



# revision 10
# speedup vs baseline: 1.1209x; 1.1209x over previous
"""ContraNorm kernel for Trainium2 (Bass/Tile), 8-core data-parallel over batch.

Reference computation per batch element b (x_b: [N=2048, D=512] fp32):
    xn   = x_b / ||x_b||_2(rows)
    sim  = xn @ xn.T                      # [N, N]
    P    = softmax(sim, axis=1)
    y    = layernorm(x_b - 0.1 * P @ x_b) * w + b

Structure exploited:
  * E = exp(sim) is SYMMETRIC, so the second matmul (P @ x) uses E's
    natural-layout tiles as the stationary matmul operand.
  * sim entries are cosine similarities in [-1, 1]: no max-subtraction
    needed, and xn is small/bounded -> the sim matmul runs in FP8-E4M3
    with DoubleRow perf mode (2x PE throughput). The fp8 quantization
    noise averages out over the 512-long contraction.

Engine assignment (driven by trace analysis):
  * ACT is the scarce engine: it runs ONLY Square+accum / Sqrt (phase 0),
    the 32 exp strips (+accum row sums), and the phase-B Square/Sqrt.
    Function groups are kept contiguous to avoid ACT table reloads.
  * All PSUM->SBUF evictions of xn^T go to DVE (fp8 cast copies).
  * x -> bf16 cast for the phase-B rhs runs on idle GpSimd.
  * Phase-B epilogue: fused DVE ops (PSUM evict + residual + rowsum in
    one scalar_tensor_tensor), ACT Square accum for variance, fused
    (y-mean)*rstd, w on DVE, b on GpSimd.
  * x loads alternate between the sync and gpsimd DMA rings.
"""

import numpy as np
from contextlib import ExitStack

import concourse.bass as bass
import concourse.tile as tile
from concourse import bacc, mybir
from concourse.bass_utils import run_bass_kernel_spmd
from concourse.masks import make_identity

F32 = mybir.dt.float32
BF16 = mybir.dt.bfloat16
FP8 = mybir.dt.float8e4
AF = mybir.ActivationFunctionType
OP = mybir.AluOpType
DR = mybir.MatmulPerfMode.DoubleRow

B, N, D = 8, 2048, 512
P = 128                 # partitions
NT = N // P             # 16 row blocks
KD = D // P             # 4 d blocks
SCALE = 0.1
EPS = 1e-6


def build_nc(n: int = N, num_devices: int = B, use_fp8: bool = True):
    NT = n // P
    HW_ = min(n, 1024)       # psum strip width (2 banks)
    H = n // HW_             # strips per sim row
    nc = bacc.Bacc("TRN2", target_bir_lowering=False, debug=False,
                   num_devices=num_devices)

    x_d = nc.dram_tensor("x", [n, D], F32, kind="ExternalInput").ap()
    w_d = nc.dram_tensor("ln_weight", [D], F32, kind="ExternalInput").ap()
    b_d = nc.dram_tensor("ln_bias", [D], F32, kind="ExternalInput").ap()
    out_d = nc.dram_tensor("out", [n, D], F32, kind="ExternalOutput").ap()

    x_r = x_d.rearrange("(t p) d -> p t d", p=P)      # [128, 16, 512]
    out_r = out_d.rearrange("(t p) d -> p t d", p=P)

    with tile.TileContext(nc) as tc, ExitStack() as ctx:
        const = ctx.enter_context(tc.tile_pool(name="const", bufs=1))
        big = ctx.enter_context(tc.tile_pool(name="big", bufs=1))
        work = ctx.enter_context(tc.tile_pool(name="work", bufs=3))
        stats = ctx.enter_context(tc.tile_pool(name="stats", bufs=4))
        outp = ctx.enter_context(tc.tile_pool(name="outp", bufs=3))
        psT = ctx.enter_context(tc.tile_pool(name="psT", bufs=2, space="PSUM"))
        psA = ctx.enter_context(tc.tile_pool(name="psA", bufs=2, space="PSUM"))
        psB = ctx.enter_context(tc.tile_pool(name="psB", bufs=2, space="PSUM"))

        # --- constants ---
        ident = const.tile([P, P], BF16)
        make_identity(nc, ident[:])
        w_bc = const.tile([P, D], F32)
        b_bc = const.tile([P, D], F32)
        nc.gpsimd.dma_start(
            out=w_bc[:], in_=bass.AP(tensor=w_d.tensor, offset=w_d.offset,
                                     ap=[[0, P], w_d.ap[0]]))
        nc.gpsimd.dma_start(
            out=b_bc[:], in_=bass.AP(tensor=b_d.tensor, offset=b_d.offset,
                                     ap=[[0, P], b_d.ap[0]]))

        # --- persistent big buffers ---
        x_all = big.tile([P, NT, D], F32)        # x row blocks      (32KB/part)
        xb_all = big.tile([P, NT, D], BF16)      # x bf16 (B rhs)    (16KB/part)
        sim_dt = FP8 if use_fp8 else BF16
        xnT = big.tile([P, KD, n], sim_dt)       # xn^T d-blocks
        e_all = big.tile([P, NT, n], BF16)       # exp(sim)          (64KB/part)

        eps_t = const.tile([P, 1], F32)
        nc.vector.memset(eps_t[:], EPS)
        ss = const.tile([P, NT], F32)            # row sum-of-squares
        rss = const.tile([P, NT], F32)           # 1/ss
        invn = const.tile([P, NT], F32)          # 1/||row||
        s_half = const.tile([P, NT * H], F32)    # exp accum partials
        rsum = const.tile([P, NT], F32)          # softmax row sums
        r1 = const.tile([P, NT], F32)
        negr = const.tile([P, NT], F32)          # -SCALE / rsum

        # ---------------- Phase 0: per-block chains -----------------------
        def phase0_chain(i):
            eng = nc.sync if i % 2 == 0 else nc.gpsimd
            eng.dma_start(out=x_all[:, i, :], in_=x_r[:, i, :])
            sq_scratch = work.tile([P, D], BF16, tag="sq")
            nc.scalar.activation(out=sq_scratch[:], in_=x_all[:, i, :],
                                 func=AF.Square, accum_out=ss[:, i:i + 1])
            if i % 2 == 1:
                nc.vector.reciprocal(out=rss[:, i - 1:i + 1],
                                     in_=ss[:, i - 1:i + 1])
                nc.scalar.activation(out=invn[:, i - 1:i + 1],
                                     in_=rss[:, i - 1:i + 1], func=AF.Sqrt)
                for ii in (i - 1, i):
                    xn = work.tile([P, D], BF16, tag="xn")
                    nc.vector.tensor_scalar_mul(xn[:], x_all[:, ii, :],
                                                invn[:, ii:ii + 1])
                    pt = psT.tile([P, KD * P], BF16, tag="pt")
                    for k in range(KD):
                        nc.tensor.transpose(pt[:, k * P:(k + 1) * P],
                                            xn[:, k * P:(k + 1) * P], ident[:])
                    nc.vector.tensor_copy(
                        xnT[:, :, ii * P:(ii + 1) * P],
                        pt[:].rearrange("p (k m) -> p k m", k=KD))
                    # bf16 cast of x for the phase-B rhs, on idle GpSimd
                    nc.gpsimd.tensor_copy(out=xb_all[:, ii, :],
                                          in_=x_all[:, ii, :])

        # ---------------- Phase A: sim strips + exp ------------------------
        def simA(i, h):
            pa = psA.tile([P, HW_], F32, tag="pa")
            for cc in range(HW_ // 512):
                col0 = h * HW_ + cc * 512
                if use_fp8:
                    for kk in range(KD // 2):
                        nc.tensor.matmul(
                            pa[:, cc * 512:(cc + 1) * 512],
                            lhsT=xnT[:, 2 * kk:2 * kk + 2, i * P:(i + 1) * P],
                            rhs=xnT[:, 2 * kk:2 * kk + 2, col0:col0 + 512],
                            start=(kk == 0), stop=(kk == KD // 2 - 1),
                            perf_mode=DR)
                else:
                    for k in range(KD):
                        nc.tensor.matmul(
                            pa[:, cc * 512:(cc + 1) * 512],
                            lhsT=xnT[:, k, i * P:(i + 1) * P],
                            rhs=xnT[:, k, col0:col0 + 512],
                            start=(k == 0), stop=(k == KD - 1))
            nc.scalar.activation(
                out=e_all[:, i, h * HW_:(h + 1) * HW_], in_=pa[:],
                func=AF.Exp, accum_out=s_half[:, H * i + h:H * i + h + 1])

        # h=0 strips first (cols 0..1023 for every row) so the first half of
        # phase B's stationary tiles complete as early as possible. Strips
        # (0,0)/(1,0) fill both psA bufs right after their operands exist, so
        # the PE is never starved while the phase-0 chains for blocks 8..15
        # are still streaming in.
        for i in range(8):
            phase0_chain(i)
        simA(0, 0)
        simA(1, 0)
        for i in range(8, NT):
            phase0_chain(i)
        for i in range(2, NT):
            simA(i, 0)
        for i in range(NT):
            simA(i, 1)

        # negr = -SCALE / rowsum
        if H > 1:
            s_view = s_half.rearrange("p (i h) -> p i h", h=H)
            nc.vector.reduce_sum(
                rsum[:].rearrange("p (i o) -> p i o", o=1), s_view,
                axis=mybir.AxisListType.X)
        else:
            nc.vector.tensor_copy(rsum[:], s_half[:])
        nc.vector.reciprocal(out=r1[:], in_=rsum[:])
        nc.vector.tensor_scalar_mul(negr[:], r1[:], -SCALE)

        # ---------------- Phase B: x_neg + residual + layernorm ------------
        for i in range(NT):
            pb = psB.tile([P, D], F32, tag="pb")
            for j in range(NT):
                nc.tensor.matmul(
                    pb[:], lhsT=e_all[:, j, i * P:(i + 1) * P],
                    rhs=xb_all[:, j, :], start=(j == 0), stop=(j == NT - 1))

            # y0 = x + negr*pb ; s1 = rowsum(y0)   (single fused DVE op)
            y0 = work.tile([P, D], F32, tag="y0")
            s1 = stats.tile([P, 1], F32, tag="s1")
            nc.vector.scalar_tensor_tensor(out=y0[:], in0=pb[:],
                                           scalar=negr[:, i:i + 1],
                                           in1=x_all[:, i, :],
                                           op0=OP.mult, op1=OP.add,
                                           accum_out=s1[:])
            sqd = work.tile([P, D], BF16, tag="sqd")
            ssq = stats.tile([P, 1], F32, tag="ssq")
            nc.scalar.activation(out=sqd[:], in_=y0[:], func=AF.Square,
                                 accum_out=ssq[:])
            mean = stats.tile([P, 1], F32, tag="mean")
            nc.vector.tensor_scalar_mul(mean[:], s1[:], 1.0 / D)
            m2 = stats.tile([P, 1], F32, tag="m2")
            nc.vector.tensor_mul(m2[:], mean[:], mean[:])
            var = stats.tile([P, 1], F32, tag="var")
            nc.vector.scalar_tensor_tensor(out=var[:], in0=ssq[:],
                                           scalar=1.0 / D, in1=m2[:],
                                           op0=OP.mult, op1=OP.subtract)
            rstd = stats.tile([P, 1], F32, tag="rstd")
            nc.scalar.activation(out=rstd[:], in_=var[:], func=AF.Sqrt,
                                 bias=eps_t[:], scale=1.0)  # sqrt(var+eps)
            nc.vector.reciprocal(out=rstd[:], in_=rstd[:])

            # z = (y0 - mean) * rstd   (fused)
            z = work.tile([P, D], F32, tag="z")
            nc.vector.tensor_scalar(out=z[:], in0=y0[:],
                                    scalar1=mean[:, 0:1], scalar2=rstd[:, 0:1],
                                    op0=OP.subtract, op1=OP.mult)
            zw = work.tile([P, D], F32, tag="zw")
            nc.vector.tensor_mul(zw[:], z[:], w_bc[:])
            yout = outp.tile([P, D], F32, tag="yout")
            nc.gpsimd.tensor_add(yout[:], zw[:], b_bc[:])
            nc.sync.dma_start(out=out_r[:, i, :], in_=yout[:])

    nc.compile()
    return nc


_NC = None


def _get_nc():
    global _NC
    if _NC is None:
        _NC = build_nc()
    return _NC


def _run(inputs: dict, trace: bool = False, **kw):
    x = np.asarray(inputs["x"], dtype=np.float32)
    w = np.asarray(inputs["ln_weight"], dtype=np.float32)
    bb = np.asarray(inputs["ln_bias"], dtype=np.float32)
    assert x.shape == (B, N, D), x.shape
    nc = _get_nc()
    in_maps = [
        {"x": np.ascontiguousarray(x[b]), "ln_weight": w, "ln_bias": bb}
        for b in range(B)
    ]
    res = run_bass_kernel_spmd(nc, in_maps, core_ids=list(range(B)), trace=trace,
                               **kw)
    out = np.stack([res.results[b]["out"] for b in range(B)], axis=0)
    return out.astype(np.float32), res


def kernel(**inputs) -> np.ndarray:
    out, _ = _run(inputs, trace=False)
    return out
